# revision 14
# baseline (speedup 1.0000x reference)
"""BSplineSynapse Trainium2 kernel (8-core tensor-parallel over out_features).

Math: reference computes, with t = clip(|x|, 0, 1), s = 1 - t:
    w(t) = cp0*s^3 + 3*cp1*s^2*t + 3*cp2*s*t^2 + cp3*t^3   (per (o, i))
    out[b, o] = sum_i w[o, i](t[b, i]) * x[b, i]

Rewritten in the monomial basis of t with the factor-3 folded into the
moving side (g1 = 3 t x, g2 = 3 t^2 x, g3 = 3 t^4... = 3 t^3 x):
    out = x @ w0^T + g1 @ A^T + g2 @ D2^T + g3 @ D3^T
    A  = w1 - w0
    D2 = w0 - 2*w1 + w2
    D3 = w3/3 - w0/3 + w1 - w2

Engine assignment (fast path, valid when 0 <= x <= 1 so t == x):
  - ScalarE: g1 = 3x^2 = Square(sqrt3*x); g3 = 3x^4 = Square(g1/sqrt3)
  - VectorE: g2 = x * g1
  - TensorE computes A/D2/D3 as matmuls with scaled-identity stationary
    tiles accumulated in PSUM (psum = sum_j alpha_j * w_j), freeing the
    vector engine; results are copied PSUM->SBUF (f32r) for use as lhsT.
  - 32 accumulating f32r matmuls (4 bases x 8 K=128-chunks, N=512) into
    one PSUM bank -> out^T slice (128, 512) per core.

x and the cp_k^T slices are pre-permuted on host into SBUF layout so every
DMA is a plain contiguous (128, N) copy at full bandwidth:
  x:   [p, c*512 + b] = x[b, c*128 + p], split in two halves (c 0-3 / 4-7)
  w_k: [p, c*128 + o] = cp_k[o + 128*core, c*128 + p]

A general fallback path computes t = clip(|x|, 0, 1) explicitly. Path
choice only inspects the input range; both paths implement the full
reference function on device.
"""

import sys

if "/opt/trn_rl_repo" not in sys.path:
    sys.path.insert(0, "/opt/trn_rl_repo")

import numpy as np

import concourse.bacc as bacc
import concourse.mybir as mybir
from concourse.mybir import ActivationFunctionType as AF
from concourse.mybir import AluOpType as alu
from concourse.tile import TileContext
from concourse.bass_utils import run_bass_kernel_spmd
from concourse.masks import make_identity

B = 512           # batch
I = 1024          # in_features
O = 1024          # out_features
NCORES = 8
OS = O // NCORES  # out_features per core = 128
CH = I // 128     # i-chunks of 128 = 8
HB = (CH // 2) * B  # x free-dim columns per half = 2048
WC = CH * OS      # weight free-dim columns = 1024

F32 = mybir.dt.float32
F32R = mybir.dt.float32r
SQ3 = 3.0 ** 0.5

_programs = {}


def _build(fast: bool):
    nc = bacc.Bacc("TRN2", target_bir_lowering=False, debug=False)
    xd = [
        nc.dram_tensor(f"x{h}", [128, HB], F32, kind="ExternalInput")
        for h in range(2)
    ]
    wd = [
        nc.dram_tensor(f"w{k}", [128, WC], F32, kind="ExternalInput")
        for k in range(4)
    ]
    outT = nc.dram_tensor("outT", [OS, B], F32, kind="ExternalOutput")

    with TileContext(nc) as tc:
        with (
            tc.tile_pool(name="p", bufs=1) as pool,
            tc.tile_pool(name="ps", bufs=1, space="PSUM") as pp,
        ):
            # scaled-identity stationary tiles, generated on device:
            # gpsimd writes f32 diag blocks, one ACT copy rounds to f32r
            CVALS = (1.0, -1.0, -2.0, 1.0 / 3.0, -1.0 / 3.0)
            craw = pool.tile([128, 5 * 128], F32, tag="craw", name="craw")
            nc.gpsimd.memset(craw[:], 0.0)
            for j, val in enumerate(CVALS):
                nc.gpsimd.affine_select(
                    out=craw[:, j * 128:(j + 1) * 128],
                    in_=craw[:, j * 128:(j + 1) * 128],
                    compare_op=alu.not_equal,
                    fill=val,
                    base=0,
                    pattern=[[-1, 128]],
                    channel_multiplier=1,
                )
            cblk = pool.tile([128, 5 * 128], F32R, tag="cblk", name="cblk")
            nc.scalar.copy(cblk[:], craw[:])
            consts = {
                nm: cblk[:, j * 128:(j + 1) * 128]
                for j, nm in enumerate(
                    ("cpos", "cneg", "cneg2", "cthird", "cnthird")
                )
            }

            # input DMAs, in arrival-priority order: xA, w0, w1, xB, w2, w3
            xs = [
                pool.tile([128, HB], F32R, tag=f"x{h}", name=f"x{h}")
                for h in range(2)
            ]
            w_sb = [
                pool.tile([128, WC], F32R, tag=f"w{k}", name=f"w{k}")
                for k in range(4)
            ]
            nc.sync.dma_start(out=xs[0][:], in_=xd[0].ap().bitcast(F32R))
            nc.sync.dma_start(out=w_sb[0][:], in_=wd[0].ap().bitcast(F32R))
            nc.sync.dma_start(out=w_sb[1][:], in_=wd[1].ap().bitcast(F32R))
            nc.sync.dma_start(out=xs[1][:], in_=xd[1].ap().bitcast(F32R))
            nc.gpsimd.dma_start(out=w_sb[2][:], in_=wd[2].ap().bitcast(F32R))
            nc.gpsimd.dma_start(out=w_sb[3][:], in_=wd[3].ap().bitcast(F32R))

            # x-side basis tensors, per half
            g1 = [pool.tile([128, HB], F32R, tag=f"g1{h}", name=f"g1{h}") for h in range(2)]
            g2 = [pool.tile([128, HB], F32R, tag=f"g2{h}", name=f"g2{h}") for h in range(2)]
            g3 = [pool.tile([128, HB], F32R, tag=f"g3{h}", name=f"g3{h}") for h in range(2)]
            if fast:
                # g1 = 3x^2, g3 = (g1/sqrt3)^2 = 3x^4, g2 = x*g1 = 3x^3
                nc.scalar.activation(g1[0][:], xs[0][:], AF.Square, scale=SQ3)
                nc.scalar.activation(g1[1][:], xs[1][:], AF.Square, scale=SQ3)
                nc.vector.tensor_mul(g2[0][:], xs[0][:], g1[0][:])
                nc.vector.tensor_mul(g2[1][:], xs[1][:], g1[1][:])
                nc.scalar.activation(g3[0][:], g1[0][:], AF.Square, scale=1.0 / SQ3)
                nc.scalar.activation(g3[1][:], g1[1][:], AF.Square, scale=1.0 / SQ3)
            else:
                for h in range(2):
                    tt = pool.tile([128, HB], F32, tag=f"tt{h}", name=f"tt{h}")
                    t2 = pool.tile([128, HB], F32, tag=f"t2{h}", name=f"t2{h}")
                    # t = clip(|x|, 0, 1)
                    nc.vector.tensor_scalar(
                        tt[:], xs[h][:], 0.0, 1.0, alu.abs_max, alu.min
                    )
                    nc.scalar.activation(t2[:], tt[:], AF.Square)
                    nc.vector.scalar_tensor_tensor(
                        g1[h][:], tt[:], 3.0, xs[h][:], alu.mult, alu.mult
                    )
                    nc.vector.scalar_tensor_tensor(
                        g2[h][:], t2[:], 3.0, xs[h][:], alu.mult, alu.mult
                    )
                    nc.vector.tensor_mul(g3[h][:], t2[:], g1[h][:])

            # transformed weights via TensorE: psum = sum_j alpha_j * w_j
            A_sb = pool.tile([128, WC], F32R, tag="A_sb", name="A_sb")
            D2_sb = pool.tile([128, WC], F32R, tag="D2_sb", name="D2_sb")
            D3_sb = pool.tile([128, WC], F32R, tag="D3_sb", name="D3_sb")
            ps_A = pp.tile([128, WC], F32, name="ps_A")
            ps_D2 = pp.tile([128, WC], F32, name="ps_D2")
            ps_D3 = pp.tile([128, WC], F32, name="ps_D3")
            psum = pp.tile([128, B], F32, name="psum")

            TRANSFORMS = [
                (ps_A, A_sb, [("cpos", 1), ("cneg", 0)]),
                (ps_D2, D2_sb, [("cpos", 0), ("cneg2", 1), ("cpos", 2)]),
                (ps_D3, D3_sb,
                 [("cthird", 3), ("cnthird", 0), ("cneg", 2), ("cpos", 1)]),
            ]

            def emit_transform_mms(ps, terms, h):
                sl = slice(h * 512, (h + 1) * 512)
                for i, (cn, k) in enumerate(terms):
                    nc.tensor.matmul(
                        ps[:, sl],
                        lhsT=consts[cn],
                        rhs=w_sb[k][:, sl],
                        start=(i == 0),
                        stop=(i == len(terms) - 1),
                    )

            G = [xs, g1, g2, g3]
            D = [w_sb[0], A_sb, D2_sb, D3_sb]

            mm_n = [0]

            def emit_main_wave(k, h):
                # 4 accumulating matmuls: bases k, x-half h (i-chunks 4h..4h+3)
                for c in range(4):
                    nc.tensor.matmul(
                        psum[:],
                        lhsT=D[k][:, (h * 4 + c) * OS:(h * 4 + c + 1) * OS],
                        rhs=G[k][h][:, c * B:(c + 1) * B],
                        start=(mm_n[0] == 0),
                        stop=(mm_n[0] == 31),
                    )
                    mm_n[0] += 1

            # PE warmup: idle bf16 matmuls on a memset scratch tile to lift
            # the HAM clock gate before real work arrives (results never
            # read; ps_A is cleared by the A transform's start=True later).
            # bf16 so it needs no f32r-rounded producer and starts at ~7us.
            wsc = pool.tile([128, 512], mybir.dt.bfloat16, tag="wsc", name="wsc")
            nc.gpsimd.memset(wsc[:], 1.0)
            for i in range(20):
                nc.tensor.matmul(
                    ps_A[:, 0:512],
                    lhsT=wsc[:, 0:128],
                    rhs=wsc[:],
                    start=(i == 0),
                    stop=(i == 19),
                )

            # PE program order ~ dependency readiness order (w2/w3 land
            # early via the gpsimd DMA stream, so the D2/D3 transform
            # matmuls fill the gap while xB is still in flight)
            emit_main_wave(0, 0)                       # needs xA, w0
            # A = w1 - w0 on DVE (idle window there; saves 4 PE matmuls)
            nc.vector.tensor_sub(A_sb[:], w_sb[1][:], w_sb[0][:])
            for h in range(2):                         # D2: needs w2
                emit_transform_mms(ps_D2, TRANSFORMS[1][2], h)
            for h in range(2):                         # D3: needs w3
                emit_transform_mms(ps_D3, TRANSFORMS[2][2], h)
            nc.vector.tensor_copy(D2_sb[:], ps_D2[:])
            for h in range(2):
                nc.vector.tensor_copy(
                    D3_sb[:, h * 512:(h + 1) * 512],
                    ps_D3[:, h * 512:(h + 1) * 512],
                )
            emit_main_wave(0, 1)                       # needs xB
            emit_main_wave(1, 0)                       # needs A_sb, g1A
            emit_main_wave(1, 1)                       # needs g1B
            emit_main_wave(2, 0)                       # needs D2_sb, g2A
            emit_main_wave(2, 1)                       # needs g2B
            emit_main_wave(3, 0)                       # needs D3_sb h0, g3A
            emit_main_wave(3, 1)                       # needs g3B

            osb = pool.tile([128, B], F32, tag="osb", name="osb")
            nc.scalar.copy(osb[:], psum[:])
            nc.sync.dma_start(out=outT.ap(), in_=osb[:])

    nc.compile()
    return nc


def _get_program(fast: bool):
    if fast not in _programs:
        _programs[fast] = _build(fast)
    return _programs[fast]


def _stage_x(x):
    # [p, c*512+b] = x[b, c*128+p]; split into halves (chunks 0-3 / 4-7)
    xt = x.T.reshape(CH, 128, B).transpose(1, 0, 2).reshape(128, CH * B)
    return (
        np.ascontiguousarray(xt[:, :HB]),
        np.ascontiguousarray(xt[:, HB:]),
    )


def _stage_w(cp, core):
    # [p, c*128+o] = cp[o + OS*core, c*128+p]
    sl = cp[core * OS:(core + 1) * OS].T  # (1024, 128) [i, o]
    return np.ascontiguousarray(
        sl.reshape(CH, 128, OS).transpose(1, 0, 2).reshape(128, WC)
    )


def make_in_maps(inputs):
    x = np.ascontiguousarray(np.asarray(inputs["x"], dtype=np.float32))
    cps = [
        np.ascontiguousarray(np.asarray(inputs[f"cp{k}"], dtype=np.float32))
        for k in range(4)
    ]
    xA, xB = _stage_x(x)
    in_maps = []
    for c in range(NCORES):
        m = {"x0": xA, "x1": xB}
        for k in range(4):
            m[f"w{k}"] = _stage_w(cps[k], c)
        in_maps.append(m)
    return in_maps


def kernel(**inputs) -> np.ndarray:
    x = np.asarray(inputs["x"], dtype=np.float32)
    fast = bool(x.min() >= 0.0) and bool(x.max() <= 1.0)
    nc = _get_program(fast)
    in_maps = make_in_maps(inputs)
    res = run_bass_kernel_spmd(nc, in_maps, core_ids=list(range(NCORES)))
    outT = np.concatenate(
        [res.results[c]["outT"] for c in range(NCORES)], axis=0
    )
    return np.ascontiguousarray(outT.T)
